# revision 15
# baseline (speedup 1.0000x reference)
"""Trainium2 Bass kernel for nn_Conv3DFusionModule (sparse 27-point gather-conv chain).

Strategy (8 NeuronCores, SPMD):
  - Voxels row-sharded 25000/core (padded to 25088 = 196*128).
  - Only ~1.6 of 27 neighbors are valid (self always valid). The self term of each
    3^3 conv is a dense matmul; the ~15k non-self valid pairs per core per conv layer
    are handled via: selective AllToAll row exchange of pre-BN activations ->
    dma_gather into k-grouped tiles -> PE transpose -> fused BN+ReLU (scalar engine)
    -> per-offset matmul -> dma_scatter_add into conflict-free slot accumulators ->
    merge (PE transpose + self-term matmul accumulated in PSUM).
  - BatchNorm: per-core bn_stats/bn_aggr, tiny AllReduce of (sum, sumsq), BN+ReLU
    fused into consumers as activation(Relu, scale, bias) with per-partition vectors.
  - Activations kept channel-major in a single SBUF buffer, overwritten in place.
All indices/permutations/padding are precomputed on the host from `nbr` (static data).
"""
import math
import os
import sys

import numpy as np

try:
    import concourse.bass as bass  # noqa
except ImportError:
    sys.path.insert(0, "/opt/trn_rl_repo")
    import concourse.bass as bass  # noqa

import concourse.bacc as bacc
import concourse.tile as tile
from concourse import mybir
from concourse.masks import make_identity

NCORES = 8
N = 200000
SH = N // NCORES          # 25000 rows per core
SHP = 25088               # padded to 196 tiles of 128
TPC = SHP // 128          # 196
HROWS = SHP // 2          # 12544 rows per y_acc half
HALF_T = TPC // 2         # 98 tiles per half
EPS = 1e-5
F32 = mybir.dt.float32
I16 = mybir.dt.int16
KS = [k for k in range(27) if k != 13]   # the 26 non-self offsets


def _ceil_to(x, m):
    return ((x + m - 1) // m) * m


def _wrap_idx(idx):
    """int16 index list -> [128, n/16] wrapped layout (entry i at [i%16, i//16]),
    replicated across the 8 gpsimd cores' partition groups."""
    n = idx.shape[0]
    assert n % 16 == 0
    t = np.zeros((128, n // 16), np.int16)
    t[:16] = idx.reshape(n // 16, 16).T
    for q in range(1, 8):
        t[16 * q:16 * q + 16] = t[:16]
    return t


def _host_prep(nbr):
    nbr = np.asarray(nbr)
    assert nbr.shape == (N, 27)
    assert (nbr[:, 13] == np.arange(N, dtype=nbr.dtype)).all(), "k=13 must be self"

    # per-core, per-k valid pair lists (i_local sorted), and slot-number per pair
    pairs = []           # pairs[c][k26] = (i_local asc, j_global)
    snums = []           # snums[c][k26] = slot number of each pair within its target
    for c in range(NCORES):
        lo, hi = c * SH, (c + 1) * SH
        sub = nbr[lo:hi]
        cnt = np.zeros(SH, np.int64)
        pk, sk = [], []
        for k in KS:
            j = sub[:, k]
            sel = np.nonzero(j >= 0)[0]
            jj = j[sel].astype(np.int64)
            sk.append(cnt[sel].copy())
            cnt[sel] += 1
            pk.append((sel, jj))
        pairs.append(pk)
        snums.append(sk)
    max_slot = max(int(s.max()) if len(s) else 0 for sk in snums for s in sk)

    # ---- exchange metadata (shared by the L3 and L6 convs; L1 is host-pregathered)
    rows = [[None] * NCORES for _ in range(NCORES)]   # rows[d][s] = sorted local j
    for d in range(NCORES):
        alljs = np.concatenate([jj for (_, jj) in pairs[d]])
        for s in range(NCORES):
            in_s = alljs[(alljs >= s * SH) & (alljs < (s + 1) * SH)] - s * SH
            rows[d][s] = np.unique(in_s)
    S = _ceil_to(max(max(len(rows[d][s]) for s in range(NCORES)) for d in range(NCORES)), 32)
    assert NCORES * S < 32768, f"A2A buffer rows {NCORES*S} exceed int16"

    send_idx = []
    for s in range(NCORES):
        li = np.zeros(NCORES * S, np.int16)
        for d in range(NCORES):
            r = rows[d][s]
            li[d * S:d * S + len(r)] = r.astype(np.int16)
        send_idx.append(li)

    # ---- k-tile layout (shared across conv layers)
    ntile_k = []
    for ki, k in enumerate(KS):
        mx = max(len(pairs[c][ki][0]) for c in range(NCORES))
        ntile_k.append(max(1, math.ceil(mx / 128)))
    half_ks = [list(range(13)), list(range(13, 26))]
    ktiles = []          # flat list of (half, ki, slot_off)
    k_off = {}
    off = 0
    Nh = []
    for h, kl in enumerate(half_ks):
        h0 = off
        for ki in kl:
            k_off[ki] = off
            for t in range(ntile_k[ki]):
                ktiles.append((h, ki, off + t * 128))
            off += ntile_k[ki] * 128
        Nh.append(off - h0)
    Np = off
    assert Np < 32768

    # per-core slot tables
    recv_idx, g1cols, slot_tab = [], [], []
    for c in range(NCORES):
        rid = np.zeros(Np, np.int16)
        g1 = np.full(Np, -1, np.int64)
        st = np.full((Np, 2), -1, np.int64)      # (i_local, snum)
        for ki in range(26):
            il, jj = pairs[c][ki]
            sn = snums[c][ki]
            o = k_off[ki]
            nn = len(il)
            if nn:
                src = jj // SH
                pos = np.empty(nn, np.int64)
                for s in np.unique(src):
                    m = src == s
                    pos[m] = np.searchsorted(rows[c][s], jj[m] - s * SH)
                rid[o:o + nn] = (src * S + pos).astype(np.int16)
                g1[o:o + nn] = jj
                st[o:o + nn, 0] = il
                st[o:o + nn, 1] = sn
        recv_idx.append(rid)
        g1cols.append(g1)
        slot_tab.append(st)

    # ---- scatter groups: (half, slot) with global max>0
    groups = []
    for h in range(2):
        for s in range(max_slot + 1):
            cnts = []
            for c in range(NCORES):
                st = slot_tab[c]
                m = (st[:, 1] == s) & (st[:, 0] >= 0) & ((st[:, 0] // HROWS) == h)
                cnts.append(int(m.sum()))
            Mg = _ceil_to(max(cnts), 16)
            if Mg:
                groups.append((h, s, Mg, _ceil_to(Mg, 128)))

    perm_idx, scat_idx = [], []
    for c in range(NCORES):
        st = slot_tab[c]
        pi, si = [], []
        for (h, s, Mg, Pg) in groups:
            m = (st[:, 1] == s) & (st[:, 0] >= 0) & ((st[:, 0] // HROWS) == h)
            slots = np.nonzero(m)[0]
            p = np.zeros(Pg, np.int16)
            q = np.full(Pg, 0, np.int16)
            p[:len(slots)] = slots.astype(np.int16)
            q[:len(slots)] = (st[slots, 0] % HROWS).astype(np.int16)
            # host pads (len..Mg) -> dump row
            q[len(slots):Mg] = HROWS
            pi.append(p)
            si.append(q)
        perm_idx.append(pi)
        scat_idx.append(si)

    return dict(S=S, Np=Np, Nh=Nh, ktiles=ktiles, groups=groups,
                send_idx=send_idx, recv_idx=recv_idx, g1cols=g1cols,
                perm_idx=perm_idx, scat_idx=scat_idx)


# ---------------------------------------------------------------------------
# device program
# ---------------------------------------------------------------------------
LAYER_C = dict(l1=(96, 64), l2=(64, 64), l3=(64, 64), l4=(64, 64),
               l5=(64, 96), l6=(96, 128), l7=(128, 128), l8=(128, 128))


def _emit(tc, t, meta):
    nc = tc.nc
    AF = mybir.ActivationFunctionType
    AL = mybir.AluOpType
    S, Np, Nh = meta["S"], meta["Np"], meta["Nh"]
    ktiles, groups = meta["ktiles"], meta["groups"]
    NhT = [Nh[0] // 128, Nh[1] // 128]

    glob = tc.alloc_tile_pool(name="glob", bufs=1)
    ident = glob.tile([128, 128], F32)
    make_identity(nc, ident[:])
    buf = glob.tile([128, SHP], F32)          # channel-major activations
    stats = glob.tile([128, 49 * 6], F32)
    sbv = {}                                  # layer -> (scale[C,1], bias[C,1])

    # indices to SBUF
    def load_idx(name, shape):
        til = glob.tile(shape, I16, name=name + "_sb")
        nc.sync.dma_start(til[:], t[name][:])
        return til

    send_sb = load_idx("send_idx", [128, NCORES * S // 16])
    recv_sb = load_idx("recv_idx", [128, Np // 16])
    perm_sb = [load_idx(f"perm{gi}", [128, g[3] // 16]) for gi, g in enumerate(groups)]
    scat_sb = [load_idx(f"scat{gi}", [128, g[3] // 16]) for gi, g in enumerate(groups)]

    # ---- helpers ----------------------------------------------------------
    def bn_finalize(lname, C, gname, bname):
        """stats region -> (scale, bias) tiles for layer lname."""
        with tc.tile_pool(name=f"bn_{lname}", bufs=1) as p:
            mv = p.tile([128, 2], F32)
            nc.gpsimd.memset(mv[:], 0.0)
            nc.vector.bn_aggr(mv[:C, :], stats[:C, :])
            sums = p.tile([128, 2], F32)
            nc.gpsimd.memset(sums[:], 0.0)
            # sum = mean*SHP ; sumsq = (var + mean^2)*SHP
            m2 = p.tile([128, 1], F32)
            nc.vector.tensor_tensor(m2[:C], mv[:C, 0:1], mv[:C, 0:1], op=AL.mult)
            nc.vector.tensor_tensor(sums[:C, 1:2], mv[:C, 1:2], m2[:C], op=AL.add)
            nc.vector.tensor_scalar_mul(sums[:C, 1:2], sums[:C, 1:2], float(SHP))
            nc.vector.tensor_scalar_mul(sums[:C, 0:1], mv[:C, 0:1], float(SHP))
            nc.sync.dma_start(t[f"arin_{lname}"][:], sums[:])
            nc.gpsimd.collective_compute(
                "AllReduce", AL.add, replica_groups=[list(range(NCORES))],
                ins=[t[f"arin_{lname}"][:]], outs=[t[f"arout_{lname}"][:]])
            gs = p.tile([128, 2], F32)
            nc.sync.dma_start(gs[:], t[f"arout_{lname}"][:])
            gw = p.tile([128, 1], F32)
            bw = p.tile([128, 1], F32)
            nc.sync.dma_start(gw[:C], t[gname][:])
            nc.sync.dma_start(bw[:C], t[bname][:])
            mean = p.tile([128, 1], F32)
            var = p.tile([128, 1], F32)
            nc.vector.tensor_scalar_mul(mean[:C], gs[:C, 0:1], 1.0 / N)
            nc.vector.tensor_scalar_mul(var[:C], gs[:C, 1:2], 1.0 / N)
            nc.vector.tensor_tensor(m2[:C], mean[:C], mean[:C], op=AL.mult)
            nc.vector.tensor_tensor(var[:C], var[:C], m2[:C], op=AL.subtract)
            nc.vector.tensor_scalar_add(var[:C], var[:C], EPS)
            sd = p.tile([128, 1], F32)
            nc.scalar.sqrt(sd[:C], var[:C])
            inv = p.tile([128, 1], F32)
            nc.vector.reciprocal(inv[:C], sd[:C])
            scl = glob.tile([128, 1], F32, name=f"scl_{lname}")
            bia = glob.tile([128, 1], F32, name=f"bia_{lname}")
            nc.vector.tensor_tensor(scl[:C], inv[:C], gw[:C], op=AL.mult)
            sxm = p.tile([128, 1], F32)
            nc.vector.tensor_tensor(sxm[:C], scl[:C], mean[:C], op=AL.mult)
            nc.vector.tensor_tensor(bia[:C], bw[:C], sxm[:C], op=AL.subtract)
            sbv[lname] = (scl, bia)

    def conv_pair_phase(lname, Cin, Cout, wn_name, u_name, prev_sb, pad_cin):
        """non-self pair matmuls -> u_dram. prev_sb=(scale,bias) for BN of gathered
        rows (None for l1). pad_cin: gathered row channel count (Cin padded)."""
        with tc.tile_pool(name=f"pm_{lname}", bufs=1) as wp, \
             tc.tile_pool(name=f"pp_{lname}", bufs=2, space="PSUM") as pp, \
             tc.tile_pool(name=f"ps_{lname}", bufs=3) as sp:
            wn = wp.tile([Cin, 26 * Cout], F32)
            nc.sync.dma_start(wn[:], t[wn_name][:])
            for h in (0, 1):
                nh = Nh[h]
                h_off = Nh[0] if h else 0
                tiles = [kt for kt in ktiles if kt[0] == h]
                if lname == "l1":
                    gsb = wp.tile([96, max(Nh)], F32, tag="g1half")
                    nc.sync.dma_start(gsb[:, :nh], t["G1T"][:, h_off:h_off + nh])
                else:
                    rtiles = wp.tile([128, max(NhT), pad_cin], F32, tag="rtiles")
                    nc.gpsimd.dma_gather(
                        out_ap=rtiles[:, :nh // 128, :], in_ap=t["a2a_out_" + lname][:],
                        idxs_ap=recv_sb[:, h_off // 16:(h_off + nh) // 16],
                        num_idxs=nh, num_idxs_reg=nh, elem_size=pad_cin, single_packet=False)
                for g0 in range(0, len(tiles), 4):
                    gtl = tiles[g0:g0 + 4]
                    ng = len(gtl)
                    pu = pp.tile([128, 4 * Cout], F32, tag="pu")
                    if lname != "l1":
                        pt = pp.tile([128, 4 * 128], F32, tag="pt")
                        gt = sp.tile([Cin, 4 * 128], F32, tag="gt")
                        for j, (_, ki, soff) in enumerate(gtl):
                            ti = (soff - h_off) // 128
                            nc.tensor.matmul(
                                pt[:pad_cin, j * 128:(j + 1) * 128],
                                rtiles[:, ti, :], ident[:],
                                is_transpose=True, start=(j == 0), stop=(j == ng - 1))
                        sc, bi = prev_sb
                        nc.scalar.activation(gt[:, :ng * 128], pt[:Cin, :ng * 128],
                                             AF.Relu, bias=bi[:Cin], scale=sc[:Cin])
                    for j, (_, ki, soff) in enumerate(gtl):
                        if lname == "l1":
                            lhs = gsb[:, soff - h_off:soff - h_off + 128]
                        else:
                            lhs = gt[:, j * 128:(j + 1) * 128]
                        nc.tensor.matmul(pu[:, j * Cout:(j + 1) * Cout], lhs,
                                         wn[:, ki * Cout:(ki + 1) * Cout],
                                         start=(j == 0), stop=(j == ng - 1))
                    ust = sp.tile([128, 4, Cout], F32, tag="ust")
                    nc.scalar.copy(ust[:, :ng, :], pu[:, :ng * Cout].rearrange("p (j c) -> p j c", c=Cout))
                    s0 = gtl[0][2]
                    dst = t[u_name][s0:s0 + ng * 128, :]
                    nc.sync.dma_start(dst.rearrange("(j p) c -> p j c", p=128), ust[:, :ng, :])

    def scatter_phase(lname, Cout, u_name, ya_names):
        with tc.tile_pool(name=f"sc_{lname}", bufs=2) as sp:
            for gi, (h, s, Mg, Pg) in enumerate(groups):
                gt = sp.tile([128, max(g[3] for g in groups) // 128, Cout], F32, tag="sg")
                nc.gpsimd.dma_gather(
                    out_ap=gt[:, :Pg // 128, :], in_ap=t[u_name][:],
                    idxs_ap=perm_sb[gi][:, :Mg // 16], num_idxs=Mg, num_idxs_reg=Mg,
                    elem_size=Cout, single_packet=False)
                nc.gpsimd.dma_scatter_add(
                    out_ap=t[ya_names[h]][:], in_ap=gt[:, :Pg // 128, :],
                    idxs_ap=scat_sb[gi][:, :Mg // 16], num_idxs=Mg, num_idxs_reg=Mg,
                    elem_size=Cout, single_packet=False)

    def conv_merge(lname, Cin, Cout, ya_names, w13_name, rhs_fn):
        """y_acc + self-term -> buf[:Cout], bn_stats into stats region."""
        with tc.tile_pool(name=f"mg_{lname}", bufs=1) as wp, \
             tc.tile_pool(name=f"mp_{lname}", bufs=2, space="PSUM") as pp, \
             tc.tile_pool(name=f"ms_{lname}", bufs=3) as sp:
            w13 = wp.tile([Cin, Cout], F32)
            nc.sync.dma_start(w13[:], t[w13_name][:])
            for ch in range(49):
                ysb = sp.tile([128, 4, Cout], F32, tag="ysb")
                t0 = ch * 4
                # contiguous runs within one y_acc half
                runs = []
                if t0 + 4 <= HALF_T or t0 >= HALF_T:
                    runs.append((t0, 4))
                else:
                    runs.append((t0, HALF_T - t0))
                    runs.append((HALF_T, t0 + 4 - HALF_T))
                for (rt, rn) in runs:
                    h = 1 if rt >= HALF_T else 0
                    r0 = rt * 128 - h * HROWS
                    src = t[ya_names[h]][r0:r0 + rn * 128, :]
                    nc.sync.dma_start(
                        ysb[:, rt - t0:rt - t0 + rn, :],
                        src.rearrange("(j p) c -> p j c", p=128))
                ps = pp.tile([Cout, 512], F32, tag="mps")
                for j in range(4):
                    nc.tensor.matmul(ps[:, j * 128:(j + 1) * 128], ysb[:, j, :],
                                     ident[:], is_transpose=True, start=(j == 0),
                                     stop=False)
                rhs = rhs_fn(ch, sp)
                nc.tensor.matmul(ps[:], w13[:], rhs, start=False, stop=True)
                sl = slice(ch * 512, (ch + 1) * 512)
                nc.scalar.copy(buf[:Cout, sl], ps[:])
                if ch == 48:
                    nc.vector.memset(buf[:Cout, SH:SHP], 0.0)
                nc.vector.bn_stats(stats[:Cout, ch * 6:(ch + 1) * 6], buf[:Cout, sl])

    def act_tile(p, lname, Cin, sl, width, tag="ht"):
        sc, bi = sbv[lname]
        ht = p.tile([Cin, width], F32, tag=tag)
        nc.scalar.activation(ht[:], buf[:Cin, sl], AF.Relu, bias=bi[:Cin], scale=sc[:Cin])
        return ht

    def dense_layer(lname, prev, Cin, Cout, w_name, exchange_rm=None):
        with tc.tile_pool(name=f"dn_{lname}", bufs=1) as wp, \
             tc.tile_pool(name=f"dp_{lname}", bufs=2, space="PSUM") as pp, \
             tc.tile_pool(name=f"ds_{lname}", bufs=3) as sp:
            w = wp.tile([Cin, Cout], F32)
            nc.sync.dma_start(w[:], t[w_name][:])
            if exchange_rm is None:
                for ch in range(49):
                    sl = slice(ch * 512, (ch + 1) * 512)
                    ht = act_tile(sp, prev, Cin, sl, 512)
                    ps = pp.tile([Cout, 512], F32, tag="dps")
                    nc.tensor.matmul(ps[:], w[:], ht[:], start=True, stop=True)
                    nc.scalar.copy(buf[:Cout, sl], ps[:])
                    if ch == 48:
                        nc.vector.memset(buf[:Cout, SH:SHP], 0.0)
                    nc.vector.bn_stats(stats[:Cout, ch * 6:(ch + 1) * 6], buf[:Cout, sl])
            else:
                rm_name, rm_pad = exchange_rm
                for ch in range(49):
                    rst = sp.tile([128, 4, rm_pad], F32, tag="rst")
                    for j in range(4):
                        ti = ch * 4 + j
                        sl = slice(ti * 128, (ti + 1) * 128)
                        ht = act_tile(sp, prev, Cin, sl, 128)
                        pa = pp.tile([128, Cout], F32, tag="dpa")
                        nc.tensor.matmul(pa[:], ht[:], w[:], start=True, stop=True)
                        nc.vector.tensor_copy(rst[:, j, :Cout], pa[:])
                        pb = pp.tile([Cout, 128], F32, tag="dpb")
                        nc.tensor.matmul(pb[:], w[:], ht[:], start=True, stop=True)
                        nc.scalar.copy(buf[:Cout, sl], pb[:])
                    dst = t[rm_name][ch * 512:(ch + 1) * 512, 0:Cout]
                    nc.sync.dma_start(dst.rearrange("(j p) c -> p j c", p=128),
                                      rst[:, :, :Cout])
                    csl = slice(ch * 512, (ch + 1) * 512)
                    if ch == 48:
                        nc.vector.memset(buf[:Cout, SH:SHP], 0.0)
                    nc.vector.bn_stats(stats[:Cout, ch * 6:(ch + 1) * 6], buf[:Cout, csl])

    def exchange(lname, pad_c, rm_name):
        with tc.tile_pool(name=f"ex_{lname}", bufs=2) as sp:
            for h in (0, 1):
                st = sp.tile([128, (NCORES * S // 2) // 128, pad_c], F32, tag="sndt")
                o = h * (NCORES * S // 2)
                nc.gpsimd.dma_gather(
                    out_ap=st[:], in_ap=t[rm_name][:],
                    idxs_ap=send_sb[:, o // 16:(o + NCORES * S // 2) // 16],
                    num_idxs=NCORES * S // 2, num_idxs_reg=NCORES * S // 2,
                    elem_size=pad_c, single_packet=False)
                dst = t["a2a_in_" + lname][o:o + NCORES * S // 2, :]
                nc.sync.dma_start(dst.rearrange("(j p) c -> p j c", p=128), st[:])
            nc.gpsimd.collective_compute(
                "AllToAll", mybir.AluOpType.bypass,
                replica_groups=[list(range(NCORES))],
                ins=[t["a2a_in_" + lname][:]], outs=[t["a2a_out_" + lname][:]])

    # ---- the network ------------------------------------------------------
    # L1 conv (96 -> 64): pairs from host-pregathered G1T, self from xT0
    conv_pair_phase("l1", 96, 64, "Wn1", "u1", None, 96)
    scatter_phase("l1", 64, "u1", ("ya1a", "ya1b"))

    def l1_rhs(ch, sp):
        xt = sp.tile([96, 512], F32, tag="xt0")
        nc.sync.dma_start(xt[:], t["xT0"][:, ch * 512:(ch + 1) * 512])
        return xt[:]
    conv_merge("l1", 96, 64, ("ya1a", "ya1b"), "W13_1", l1_rhs)
    bn_finalize("l1", 64, "g1bn", "b1bn")

    # L2 dense 64->64 with row-major export for the exchange
    dense_layer("l2", "l1", 64, 64, "W2", exchange_rm=("z2rm", 64))
    bn_finalize("l2", 64, "g2bn", "b2bn")
    exchange("l3", 64, "z2rm")

    # L3 conv 64->64
    conv_pair_phase("l3", 64, 64, "Wn3", "u3", sbv["l2"], 64)
    scatter_phase("l3", 64, "u3", ("ya3a", "ya3b"))

    def l3_rhs(ch, sp):
        return act_tile(sp, "l2", 64, slice(ch * 512, (ch + 1) * 512), 512, tag="h2")[:]
    conv_merge("l3", 64, 64, ("ya3a", "ya3b"), "W13_3", l3_rhs)
    bn_finalize("l3", 64, "g3bn", "b3bn")

    # L4 dense 64->64
    dense_layer("l4", "l3", 64, 64, "W4")
    bn_finalize("l4", 64, "g4bn", "b4bn")

    # L5 dense 64->96 with row-major export (rows padded to 128 ch; z5rm pre-zeroed)
    dense_layer("l5", "l4", 64, 96, "W5", exchange_rm=("z5rm", 128))
    bn_finalize("l5", 96, "g5bn", "b5bn")
    exchange("l6", 128, "z5rm")

    # L6 conv 96->128
    conv_pair_phase("l6", 96, 128, "Wn6", "u6", sbv["l5"], 128)
    scatter_phase("l6", 128, "u6", ("ya6a", "ya6b"))

    def l6_rhs(ch, sp):
        return act_tile(sp, "l5", 96, slice(ch * 512, (ch + 1) * 512), 512, tag="h5")[:]
    conv_merge("l6", 96, 128, ("ya6a", "ya6b"), "W13_6", l6_rhs)
    bn_finalize("l6", 128, "g6bn", "b6bn")

    # L7, L8 dense 128->128
    dense_layer("l7", "l6", 128, 128, "W7")
    bn_finalize("l7", 128, "g7bn", "b7bn")
    dense_layer("l8", "l7", 128, 128, "W8")
    bn_finalize("l8", 128, "g8bn", "b8bn")

    # final activation -> output (channel-major; host transposes)
    with tc.tile_pool(name="out", bufs=3) as sp:
        for ch in range(49):
            sl = slice(ch * 512, (ch + 1) * 512)
            ot = act_tile(sp, "l8", 128, sl, 512, tag="ot")
            nc.sync.dma_start(t["outT"][:, sl], ot[:])

    glob.release()


# ---------------------------------------------------------------------------
# program assembly
# ---------------------------------------------------------------------------
def _declare_tensors(nc, meta):
    S, Np = meta["S"], meta["Np"]
    groups = meta["groups"]
    t = {}

    def ein(name, shape, dtype=F32):
        t[name] = nc.dram_tensor(name, shape, dtype, kind="ExternalInput").ap()

    def internal(name, shape, dtype=F32, shared=False):
        t[name] = nc.dram_tensor(
            name, shape, dtype, kind="Internal",
            addr_space="Shared" if shared else "Local").ap()

    ein("xT0", [96, SHP])
    ein("G1T", [96, Np])
    ein("send_idx", [128, NCORES * S // 16], I16)
    ein("recv_idx", [128, Np // 16], I16)
    for gi, g in enumerate(groups):
        ein(f"perm{gi}", [128, g[3] // 16], I16)
        ein(f"scat{gi}", [128, g[3] // 16], I16)
    ein("Wn1", [96, 26 * 64]); ein("W13_1", [96, 64])
    ein("Wn3", [64, 26 * 64]); ein("W13_3", [64, 64])
    ein("Wn6", [96, 26 * 128]); ein("W13_6", [96, 128])
    ein("W2", [64, 64]); ein("W4", [64, 64]); ein("W5", [64, 96])
    ein("W7", [128, 128]); ein("W8", [128, 128])
    for i, C in zip(range(1, 9), (64, 64, 64, 64, 96, 128, 128, 128)):
        ein(f"g{i}bn", [C, 1]); ein(f"b{i}bn", [C, 1])
    ein("ya1a", [HROWS + 1, 64]); ein("ya1b", [HROWS + 1, 64])
    ein("ya3a", [HROWS + 1, 64]); ein("ya3b", [HROWS + 1, 64])
    ein("ya6a", [HROWS + 1, 128]); ein("ya6b", [HROWS + 1, 128])
    ein("z5rm", [SHP, 128])      # pre-zeroed (cols 96:128 stay 0)

    internal("z2rm", [SHP, 64])
    internal("u1", [Np, 64]); internal("u3", [Np, 64]); internal("u6", [Np, 128])
    internal("a2a_in_l3", [NCORES * S, 64]); internal("a2a_out_l3", [NCORES * S, 64])
    internal("a2a_in_l6", [NCORES * S, 128]); internal("a2a_out_l6", [NCORES * S, 128])
    for ln in ("l1", "l2", "l3", "l4", "l5", "l6", "l7", "l8"):
        internal(f"arin_{ln}", [128, 2])
        internal(f"arout_{ln}", [128, 2], shared=True)

    t["outT"] = nc.dram_tensor("outT", [128, SHP], F32, kind="ExternalOutput").ap()
    return t


def _build_in_maps(inputs, meta):
    feat3d = np.asarray(inputs["feat3d"], np.float32)
    p = inputs["params"]
    S, Np = meta["S"], meta["Np"]
    groups = meta["groups"]

    def W(x):
        return np.ascontiguousarray(np.asarray(x, np.float32))

    W3d0, W3d2, Wf0 = W(p["W3d0"]), W(p["W3d2"]), W(p["Wf0"])
    shared = dict(
        Wn1=np.ascontiguousarray(W3d0[KS].transpose(1, 0, 2).reshape(96, 26 * 64)),
        W13_1=W(W3d0[13]),
        Wn3=np.ascontiguousarray(W3d2[KS].transpose(1, 0, 2).reshape(64, 26 * 64)),
        W13_3=W(W3d2[13]),
        Wn6=np.ascontiguousarray(Wf0[KS].transpose(1, 0, 2).reshape(96, 26 * 128)),
        W13_6=W(Wf0[13]),
        W2=W(p["W3d1"]), W4=W(p["W3d3"]), W5=W(p["We"]),
        W7=W(p["Wf1"]), W8=W(p["Wf2"]),
        z5rm=np.zeros((SHP, 128), np.float32),
        ya1a=np.zeros((HROWS + 1, 64), np.float32),
        ya1b=np.zeros((HROWS + 1, 64), np.float32),
        ya3a=np.zeros((HROWS + 1, 64), np.float32),
        ya3b=np.zeros((HROWS + 1, 64), np.float32),
        ya6a=np.zeros((HROWS + 1, 128), np.float32),
        ya6b=np.zeros((HROWS + 1, 128), np.float32),
    )
    for i, (gk, bk) in enumerate(
            [("g0", "b0"), ("g1", "b1"), ("g2", "b2"), ("g3", "b3"),
             ("ge", "be"), ("gf0", "bf0"), ("gf1", "bf1"), ("gf2", "bf2")], start=1):
        shared[f"g{i}bn"] = W(p[gk]).reshape(-1, 1)
        shared[f"b{i}bn"] = W(p[bk]).reshape(-1, 1)

    in_maps = []
    for c in range(NCORES):
        m = dict(shared)
        sh = feat3d[c * SH:(c + 1) * SH]
        xT0 = np.zeros((96, SHP), np.float32)
        xT0[:, :SH] = sh.T
        m["xT0"] = xT0
        g1 = np.zeros((96, Np), np.float32)
        cols = meta["g1cols"][c]
        valid = cols >= 0
        g1[:, valid] = feat3d[cols[valid]].T
        m["G1T"] = g1
        m["send_idx"] = _wrap_idx(meta["send_idx"][c])
        m["recv_idx"] = _wrap_idx(meta["recv_idx"][c])
        for gi in range(len(groups)):
            m[f"perm{gi}"] = _wrap_idx(meta["perm_idx"][c][gi])
            m[f"scat{gi}"] = _wrap_idx(meta["scat_idx"][c][gi])
        in_maps.append(m)
    return in_maps


def build_program(meta):
    nc = bacc.Bacc("TRN2", target_bir_lowering=False, debug=False,
                   num_devices=NCORES)
    with tile.TileContext(nc) as tc:
        t = _declare_tensors(nc, meta)
        _emit(tc, t, meta)
    nc.compile()
    return nc


def kernel(**inputs):
    from concourse.bass_utils import run_bass_kernel_spmd
    meta = _host_prep(np.asarray(inputs["nbr"]))
    in_maps = _build_in_maps(inputs, meta)
    nc = build_program(meta)
    res = run_bass_kernel_spmd(nc, in_maps, core_ids=list(range(NCORES)))
    out = np.empty((N, 128), np.float32)
    for c in range(NCORES):
        out[c * SH:(c + 1) * SH] = res.results[c]["outT"].T[:SH]
    return out
